# revision 15
# baseline (speedup 1.0000x reference)
"""Mamba mixer (nn_Mixer) Trainium2 Bass kernel, v3.

Sharding: tensor-parallel over d_inner (2048 -> 256 per core, 8 cores).

Structure:
  Phase 1 (per 512-token chunk): in_proj (f32r matmuls, full PE speed),
    causal conv1d as 4 diagonal-matrix PE matmuls, silu (ACT), x_proj
    partial (bf16).  Partials for each batch are DMA'd to DRAM and ONE
    AllReduce per batch (2 total) is fired from the otherwise-empty
    gpsimd queue -- collectives have ~80us fixed latency here, so fewer
    is better, and their latency overlaps phase-1/phase-2 compute.
    silu(z) and conv output xs are spilled to DRAM (SBUF pressure) and
    streamed back in phase 2.
  Phase 2 (per chunk): dt_proj (f32r), softplus, B/C broadcast to 128
    partitions via stride-0 DMA reads of a bf16 DRAM stage (keeps all
    scan elementwise ops in pure-bf16 SBUF for 2x DVE throughput),
    selective scan via hardware tensor_tensor_scan (split across Pool
    and DVE), y accumulation over states in PSUM via identity matmuls,
    gating, out_proj partial -> DRAM (host sums the row-parallel
    partials).

Queues: SP = u loads + y stores + z/xs spill; ACT-HWDGE = phase-2 loads
(xd/bc/z/xs/broadcasts); gpsimd = collectives (plus its share of scans).

Self-contained: hardcodes all shapes; only needs the concourse/bass
runtime that ships in the container.
"""

import os
import numpy as np

# Problem sizes (fixed by the problem statement)
D_MODEL = 1024
D_INNER = 2048
NSTATE = 16
DT_RANK = 64
DCONV = 4
BATCH = 2
SEQ = 4096

NCORES = 8
DS = D_INNER // NCORES          # 256 d_inner rows per core
DT2 = DS // 128                 # 2 partition tiles per core
NXD = DT_RANK + 2 * NSTATE      # 96
LC = 512                        # chunk length (tokens)
CPB = SEQ // LC                 # chunks per batch (8)
NCH = BATCH * CPB               # total chunks (16)
TOK = BATCH * SEQ


def _build_nc(fake_collective=False):
    """Build the Bass program (same SPMD program for all 8 cores).

    fake_collective=True replaces the AllReduce with a local DRAM copy so
    the program is single-core simulable (TimelineSim perf estimates).
    scan_dve_mod: every (col % scan_dve_mod == scan_dve_mod-1) scan runs
    on DVE instead of Pool (engine balance knob).
    w_pool_mod: every (col % w_pool_mod == w_pool_mod-1) w-mul runs on
    Pool instead of DVE.
    """
    import concourse.bass as bass
    import concourse.bacc as bacc
    import concourse.mybir as mybir
    import concourse.tile as tile

    f32 = mybir.dt.float32
    f32r = mybir.dt.float32r
    bf16 = mybir.dt.bfloat16
    AF = mybir.ActivationFunctionType
    OP = mybir.AluOpType

    nc = bacc.Bacc("TRN2", target_bir_lowering=False, debug=False,
                   num_devices=NCORES)

    # ---- kernel I/O (per-core shards prepared on the host) ----
    uT = nc.dram_tensor("uT", [D_MODEL, TOK], f32r, kind="ExternalInput")
    w_in = nc.dram_tensor("w_inT", [D_MODEL, 4 * 128], f32r, kind="ExternalInput")
    conv_diag = nc.dram_tensor("conv_diag", [128, DT2 * DCONV * 128], bf16,
                               kind="ExternalInput")
    conv_b = nc.dram_tensor("conv_b", [128, DT2], f32, kind="ExternalInput")
    w_xp = nc.dram_tensor("w_xpT", [DS, NXD], bf16, kind="ExternalInput")
    w_dt = nc.dram_tensor("w_dtT", [DT_RANK, DS], f32, kind="ExternalInput")
    dt_bias = nc.dram_tensor("dt_bias", [128, DT2], f32, kind="ExternalInput")
    a_neg = nc.dram_tensor("a_neg", [128, DT2 * NSTATE], f32, kind="ExternalInput")
    d_in = nc.dram_tensor("d_in", [128, DT2], f32, kind="ExternalInput")
    w_out = nc.dram_tensor("w_outT", [DS, D_MODEL], bf16, kind="ExternalInput")
    eye_d = nc.dram_tensor("eye128", [128, 128], bf16, kind="ExternalInput")
    y_part = nc.dram_tensor("y_part", [D_MODEL, TOK], bf16, kind="ExternalOutput")

    with tile.TileContext(nc) as tc:
        with (
            tc.tile_pool(name="const", bufs=1) as cpool,
            tc.tile_pool(name="u", bufs=2) as upool,
            tc.tile_pool(name="work", bufs=2) as wpool,
            tc.tile_pool(name="nwork", bufs=2) as npool,
            tc.tile_pool(name="scanio", bufs=6) as siopool,
            tc.tile_pool(name="obuf", bufs=3) as opool,
            tc.tile_pool(name="bcast", bufs=1) as bpool,
            tc.tile_pool(name="mm", bufs=2, space="PSUM") as psmm,
            tc.tile_pool(name="psy", bufs=2, space="PSUM") as psy,
            tc.tile_pool(name="dram", bufs=1, space="DRAM") as dpool,
        ):
            # ---- static weights into SBUF ----
            w_in_sb = cpool.tile([128, 8, 4 * 128], f32r)
            nc.sync.dma_start(w_in_sb[:], w_in.ap().rearrange(
                "(j p) m -> p j m", p=128))
            w_out_sb = cpool.tile([128, DT2, D_MODEL], bf16)
            nc.sync.dma_start(w_out_sb[:], w_out.ap().rearrange(
                "(k p) m -> p k m", p=128))
            w_xp_sb = cpool.tile([128, DT2, NXD], bf16)
            nc.sync.dma_start(w_xp_sb[:], w_xp.ap().rearrange(
                "(k p) m -> p k m", p=128))
            w_dt_sb = cpool.tile([DT_RANK, DS], f32)
            nc.sync.dma_start(w_dt_sb[:], w_dt.ap())
            conv_diag_sb = cpool.tile([128, DT2 * DCONV * 128], bf16)
            nc.sync.dma_start(conv_diag_sb[:], conv_diag.ap())
            conv_b_sb = cpool.tile([128, DT2], f32)
            nc.sync.dma_start(conv_b_sb[:], conv_b.ap())
            dt_bias_sb = cpool.tile([128, DT2], f32)
            nc.sync.dma_start(dt_bias_sb[:], dt_bias.ap())
            a_sb = cpool.tile([128, DT2 * NSTATE], f32)
            nc.sync.dma_start(a_sb[:], a_neg.ap())
            d_in_sb = cpool.tile([128, DT2], f32)
            nc.sync.dma_start(d_in_sb[:], d_in.ap())
            eye_sb = cpool.tile([128, 128], bf16)
            nc.sync.dma_start(eye_sb[:], eye_d.ap())

            # persistent state
            h_all = cpool.tile([128, DT2 * NSTATE, LC], bf16)
            carry = cpool.tile([128, DT2 * NSTATE], f32)

            # DRAM staging
            z_d = dpool.tile([128, DT2 * TOK], bf16, tag="z")
            xs_d = dpool.tile([128, DT2 * TOK], bf16, tag="xs")
            bcd = dpool.tile([2 * NSTATE, TOK], bf16, tag="bcd")
            ar_in = [dpool.tile([NXD, SEQ], f32, tag=f"arin{p}", name=f"arin{p}")
                     for p in range(BATCH)]
            ar_out = [dpool.tile([NXD, SEQ], f32, tag=f"arout{p}", name=f"arout{p}")
                      for p in range(BATCH)]
            z_d3 = z_d[:, :].rearrange("p (k t) -> p k t", k=DT2)
            xs_d3 = xs_d[:, :].rearrange("p (k t) -> p k t", k=DT2)

            uT_ap = uT.ap().rearrange("(j p) t -> p j t", p=128)

            # ================= Phase 1 =================
            p1s = {"x_prev": None}

            def p1_chunk(c):
                x_prev = p1s["x_prev"]
                t0 = c * LC
                u_sb = upool.tile([128, 8, LC], f32r, tag="u")
                nc.scalar.dma_start(u_sb[:], uT_ap[:, :, t0:t0 + LC])

                x_sb = wpool.tile([128, DT2, LC + DCONV - 1], bf16, tag="x")
                if c % CPB == 0:
                    nc.vector.memset(x_sb[:, :, 0:DCONV - 1], 0.0)
                else:
                    nc.vector.tensor_copy(x_sb[:, :, 0:DCONV - 1],
                                          x_prev[:, :, LC:LC + DCONV - 1])
                z_sb = wpool.tile([128, DT2, LC], bf16, tag="z")
                for mt in range(4):
                    ps = psmm.tile([128, LC], f32, tag="mm")
                    for j in range(8):
                        nc.tensor.matmul(
                            ps[:],
                            w_in_sb[:, j, 128 * mt:128 * (mt + 1)],
                            u_sb[:, j, :],
                            start=(j == 0), stop=(j == 7))
                    if mt < DT2:
                        nc.scalar.copy(
                            x_sb[:, mt, DCONV - 1:DCONV - 1 + LC], ps[:])
                    else:
                        nc.scalar.activation(z_sb[:, mt - DT2, :], ps[:],
                                             AF.Silu, bias=0.0)
                nc.sync.dma_start(z_d3[:, :, t0:t0 + LC], z_sb[:])

                # causal conv1d on PE: 4 diagonal matmuls accumulate in PSUM
                xs_sb = wpool.tile([128, DT2, LC], bf16, tag="xs")
                for dt in range(DT2):
                    cps = psy.tile([128, LC], f32, tag=f"y{dt}",
                                   name=f"convps{dt}")
                    for k in range(DCONV):
                        blk = (dt * DCONV + k) * 128
                        nc.tensor.matmul(cps[:],
                                         conv_diag_sb[:, blk:blk + 128],
                                         x_sb[:, dt, k:k + LC],
                                         start=(k == 0), stop=(k == DCONV - 1))
                    nc.scalar.activation(xs_sb[:, dt, :], cps[:],
                                         AF.Silu, bias=conv_b_sb[:, dt:dt + 1])
                nc.sync.dma_start(xs_d3[:, :, t0:t0 + LC], xs_sb[:])

                # x_proj partial
                xp_ps = psmm.tile([NXD, LC], f32, tag="mm", name="xp_ps")
                for dt in range(DT2):
                    nc.tensor.matmul(xp_ps[:], w_xp_sb[:, dt, :],
                                     xs_sb[:, dt, :],
                                     start=(dt == 0), stop=(dt == DT2 - 1))
                xq_sb = wpool.tile([NXD, LC], f32, tag="xq")
                nc.vector.tensor_copy(xq_sb[:], xp_ps[:])
                p = c // CPB
                tp = (c % CPB) * LC
                nc.scalar.dma_start(ar_in[p][:, tp:tp + LC], xq_sb[:])

                if c % CPB == CPB - 1:  # piece (batch) complete -> collective
                    if fake_collective:
                        nc.gpsimd.dma_start(ar_out[p][:], ar_in[p][:])
                    else:
                        nc.gpsimd.collective_compute(
                            "AllReduce", OP.add,
                            replica_groups=[list(range(NCORES))],
                            ins=[ar_in[p].opt()], outs=[ar_out[p].opt()])
                p1s["x_prev"] = x_sb

            # ================= Phase 2 (software-pipelined) ==============
            # stage A(c): loads + dt_proj + softplus + dtx + B/C broadcast
            # stage B(c): a_t exps + dbx muls
            # stage C(c): scans
            # stage D(c): w muls, y matmuls, carry, gate, out_proj, store
            # Emission: A0 B0 | A1 B1 C0 D0 | A2 B2 C1 D1 | ... so each
            # engine queue has chunk c+1 front-stage work before chunk c
            # back-stage work (keeps DVE/Pool from ping-ponging).
            HB = NSTATE // 2
            state = {}

            def stage_A(c):
                p = c // CPB
                t0 = c * LC
                tp = (c % CPB) * LC
                xd_sb = wpool.tile([DT_RANK, LC], f32, tag="xd",
                                   name=f"xd{c}")
                nc.scalar.dma_start(xd_sb[:], ar_out[p][0:DT_RANK, tp:tp + LC])
                bcf = wpool.tile([2 * NSTATE, LC], f32, tag="bcf",
                                 name=f"bcf{c}")
                nc.sync.dma_start(bcf[:], ar_out[p][DT_RANK:NXD, tp:tp + LC])
                bc16 = wpool.tile([2 * NSTATE, LC], bf16, tag="bc16",
                                  name=f"bc16_{c}")
                nc.vector.tensor_copy(bc16[:], bcf[:])
                nc.sync.dma_start(bcd[:, t0:t0 + LC], bc16[:])
                bcB, bcC = [], []
                for hb in range(2):
                    bB = bpool.tile([128, HB, LC], bf16, tag=f"bcB{hb}",
                                    name=f"bcB{hb}_{c}")
                    nc.sync.dma_start(
                        bB[:],
                        bcd[HB * hb:HB * (hb + 1), t0:t0 + LC].rearrange(
                            "(o a) b -> o a b", o=1).broadcast_to(
                                [128, HB, LC]))
                    bcB.append(bB)
                    bC = bpool.tile([128, HB, LC], bf16, tag=f"bcC{hb}",
                                    name=f"bcC{hb}_{c}")
                    nc.sync.dma_start(
                        bC[:],
                        bcd[NSTATE + HB * hb:NSTATE + HB * (hb + 1),
                            t0:t0 + LC].rearrange(
                            "(o a) b -> o a b", o=1).broadcast_to(
                                [128, HB, LC]))
                    bcC.append(bC)
                z_in = wpool.tile([128, DT2, LC], bf16, tag="zin",
                                  name=f"zin{c}")
                nc.scalar.dma_start(z_in[:], z_d3[:, :, t0:t0 + LC])
                xs_in = wpool.tile([128, DT2, LC], bf16, tag="xsin",
                                   name=f"xsin{c}")
                nc.scalar.dma_start(xs_in[:], xs_d3[:, :, t0:t0 + LC])

                dt_sb = wpool.tile([128, DT2, LC], bf16, tag="dt",
                                   name=f"dt{c}")
                dtx = wpool.tile([128, DT2, LC], bf16, tag="dtx",
                                 name=f"dtx{c}")
                dt_ps = psmm.tile([128, DT2, LC], f32, tag="dtmm", bufs=1,
                                  name=f"dtps{c}")
                e_t = npool.tile([128, DT2, LC], f32, tag="esp",
                                 name=f"esp{c}")
                for dt in range(DT2):
                    nc.tensor.matmul(
                        dt_ps[:, dt, :],
                        w_dt_sb[:, 128 * dt:128 * (dt + 1)],
                        xd_sb[:],
                        start=True, stop=True)
                    nc.scalar.activation(e_t[:, dt, :], dt_ps[:, dt, :],
                                         AF.Exp, bias=dt_bias_sb[:, dt:dt + 1])
                nc.scalar.activation(dt_sb[:], e_t[:], AF.Ln, bias=1.0)
                nc.vector.tensor_mul(dtx[:], dt_sb[:], xs_in[:])
                state[c] = dict(dt_sb=dt_sb, dtx=dtx, z_in=z_in,
                                xs_in=xs_in, bcB=bcB, bcC=bcC)

            def stage_B(c):
                st = state[c]
                cols = [(dt, n) for n in range(NSTATE) for dt in range(DT2)]
                a_ts, dbxs = {}, {}
                for dt, n in cols:
                    col = dt * NSTATE + n
                    a_t = siopool.tile([128, LC], bf16, tag="a",
                                       name=f"a{c}_{col}")
                    nc.scalar.activation(a_t[:], st["dt_sb"][:, dt, :],
                                         AF.Exp, bias=0.0,
                                         scale=a_sb[:, col:col + 1])
                    a_ts[col] = a_t
                for dt, n in cols:
                    col = dt * NSTATE + n
                    dbx = siopool.tile([128, LC], bf16, tag="dbx",
                                       name=f"dbx{c}_{col}")
                    dbx_eng = nc.gpsimd if col % 2 == 1 else nc.vector
                    dbx_eng.tensor_mul(dbx[:], st["dtx"][:, dt, :],
                                       st["bcB"][n // HB][:, n % HB, :])
                    dbxs[col] = dbx
                st["a_ts"], st["dbxs"] = a_ts, dbxs

            def stage_C(c):
                st = state[c]
                if c % CPB == 0:
                    nc.vector.memset(carry[:], 0.0)
                cols = [(dt, n) for n in range(NSTATE) for dt in range(DT2)]
                for dt, n in cols:
                    col = dt * NSTATE + n
                    nc.vector.tensor_tensor_scan(
                        h_all[:, col, :], st["a_ts"][col][:],
                        st["dbxs"][col][:],
                        initial=carry[:, col:col + 1],
                        op0=OP.mult, op1=OP.add)

            def stage_D(c):
                st = state.pop(c)
                t0 = c * LC
                cols = [(dt, n) for n in range(NSTATE) for dt in range(DT2)]
                y_ps = [psy.tile([128, LC], f32, tag=f"y{i}",
                                 name=f"y_ps{c}_{i}") for i in range(DT2)]
                w_ts = {}
                for dt, n in cols:
                    col = dt * NSTATE + n
                    w_t = siopool.tile([128, LC], bf16, tag="w",
                                       name=f"w{c}_{col}")
                    w_eng = nc.gpsimd if col % 2 == 0 else nc.vector
                    w_eng.tensor_mul(w_t[:], h_all[:, col, :],
                                     st["bcC"][n // HB][:, n % HB, :])
                    w_ts[col] = w_t
                for n in range(NSTATE):
                    for dt in range(DT2):
                        col = dt * NSTATE + n
                        nc.tensor.matmul(y_ps[dt][:], eye_sb[:],
                                         w_ts[col][:],
                                         start=(n == 0),
                                         stop=(n == NSTATE - 1))
                # carry for next chunk: last column of every scan output
                if c % CPB != CPB - 1:
                    for dt in range(DT2):
                        lo, hi = dt * NSTATE, (dt + 1) * NSTATE
                        nc.scalar.copy(carry[:, lo:hi],
                                       h_all[:, lo:hi, LC - 1])

                # ---- y = y_ssm + D*xs, gate with silu(z), out_proj ----
                yg = wpool.tile([128, DT2, LC], bf16, tag="yg",
                                name=f"yg{c}")
                for dt in range(DT2):
                    ys = npool.tile([128, LC], bf16, tag="ys",
                                    name=f"ys{c}_{dt}")
                    nc.vector.scalar_tensor_tensor(
                        ys[:], st["xs_in"][:, dt, :],
                        d_in_sb[:, dt:dt + 1],
                        y_ps[dt][:], op0=OP.mult, op1=OP.add)
                    nc.vector.tensor_mul(yg[:, dt, :], ys[:],
                                         st["z_in"][:, dt, :])

                for mt in range(8):
                    ps = psmm.tile([128, LC], f32, tag="mm")
                    for kt in range(DT2):
                        nc.tensor.matmul(
                            ps[:],
                            w_out_sb[:, kt, 128 * mt:128 * (mt + 1)],
                            yg[:, kt, :],
                            start=(kt == 0), stop=(kt == DT2 - 1))
                    ob = opool.tile([128, LC], bf16, tag="ob")
                    nc.scalar.copy(ob[:], ps[:])
                    nc.sync.dma_start(
                        y_part[128 * mt:128 * (mt + 1), t0:t0 + LC], ob[:])

            # ---- emission schedule ----
            # batch-0 phase 1 (fires AR0), then a 4-chunk head start on
            # batch-1 phase 1 (covers AR0 latency), then the phase-2
            # pipeline with the remaining phase-1 chunks interleaved
            # (AR1 fires inside iteration 1).
            for c in range(CPB):
                p1_chunk(c)
            for c in range(CPB, CPB + 4):
                p1_chunk(c)
            stage_A(0)
            stage_B(0)
            for c in range(NCH):
                if c < 2:
                    p1_chunk(CPB + 4 + 2 * c)
                    p1_chunk(CPB + 5 + 2 * c)
                if c + 1 < NCH:
                    stage_A(c + 1)
                stage_C(c)
                if c + 1 < NCH:
                    stage_B(c + 1)
                stage_D(c)

    nc.compile()
    return nc


_CACHED = {}


def _get_nc():
    fake = bool(int(os.environ.get("MAMBA_FAKE_AR", "0")))
    key = ("v5", fake)
    if key not in _CACHED:
        _CACHED[key] = _build_nc(fake_collective=fake)
    return _CACHED[key]


def _host_prep(inputs):
    """Slice/transpose the full inputs into per-core in_maps."""
    import ml_dtypes
    _bf = ml_dtypes.bfloat16
    f32 = np.float32
    u = np.asarray(inputs["u"], f32)
    in_proj_w = np.asarray(inputs["in_proj_w"], f32)
    conv_w = np.asarray(inputs["conv_w"], f32)
    conv_b = np.asarray(inputs["conv_b"], f32)
    x_proj_w = np.asarray(inputs["x_proj_w"], f32)
    dt_proj_w = np.asarray(inputs["dt_proj_w"], f32)
    dt_bias = np.asarray(inputs["dt_bias"], f32)
    A_log = np.asarray(inputs["A_log"], f32)
    D_in = np.asarray(inputs["D_in"], f32)
    out_proj_w = np.asarray(inputs["out_proj_w"], f32)

    uT = np.ascontiguousarray(u.reshape(TOK, D_MODEL).T)
    eye = np.eye(128, dtype=f32).astype(_bf)
    A = -np.exp(A_log)

    def fold(v):  # (256, k) -> (128, 2*k) with dtile-major columns
        v = v.reshape(DS, -1)
        return np.ascontiguousarray(
            np.concatenate([v[:128], v[128:]], axis=1))

    in_maps = []
    for k in range(NCORES):
        sl = slice(DS * k, DS * (k + 1))
        w_in_k = np.concatenate(
            [in_proj_w[sl], in_proj_w[D_INNER + DS * k:D_INNER + DS * (k + 1)]])
        cw = fold(conv_w[sl])               # [128, DT2*DCONV]
        cd = np.zeros((128, DT2 * DCONV * 128), f32)
        for dt in range(DT2):
            for kk in range(DCONV):
                blk = (dt * DCONV + kk) * 128
                np.fill_diagonal(cd[:, blk:blk + 128], cw[:, dt * DCONV + kk])
        in_maps.append({
            "uT": uT,
            "w_inT": np.ascontiguousarray(w_in_k.T),
            "conv_diag": cd.astype(_bf),
            "conv_b": fold(conv_b[sl]),
            "w_xpT": np.ascontiguousarray(x_proj_w[:, sl].T).astype(_bf),
            "w_dtT": np.ascontiguousarray(dt_proj_w[sl].T),
            "dt_bias": fold(dt_bias[sl]),
            "a_neg": fold(A[sl]),
            "d_in": fold(D_in[sl]),
            "w_outT": np.ascontiguousarray(out_proj_w[:, sl].T).astype(_bf),
            "eye128": eye,
        })
    return in_maps


LAST_RESULTS = None


def bench(inputs, iters=24, warmup=4):
    """Estimate per-execution device time: device-put the sharded inputs
    once, then dispatch the jitted NEFF repeatedly (async) and time."""
    import time
    import jax
    from jax.sharding import Mesh, PartitionSpec, NamedSharding
    from jax.experimental.shard_map import shard_map
    import concourse.mybir as mybir
    from concourse import bass2jax
    from concourse.bass2jax import _bass_exec_p, install_neuronx_cc_hook

    install_neuronx_cc_hook()
    nc = _get_nc()
    in_maps = _host_prep(inputs)

    partition_name = (nc.partition_id_tensor.name
                      if nc.partition_id_tensor else None)
    in_names, out_names, out_avals, zero_outs = [], [], [], []
    for alloc in nc.m.functions[0].allocations:
        if not isinstance(alloc, mybir.MemoryLocationSet):
            continue
        name = alloc.memorylocations[0].name
        if alloc.kind == "ExternalInput":
            if name != partition_name:
                in_names.append(name)
        elif alloc.kind == "ExternalOutput":
            shape = tuple(alloc.tensor_shape)
            dtype = mybir.dt.np(alloc.dtype)
            out_avals.append(jax.core.ShapedArray(shape, dtype))
            out_names.append(name)
            zero_outs.append(np.zeros(shape, dtype))
    n_params = len(in_names)
    all_in_names = list(in_names) + list(out_names)
    if partition_name is not None:
        all_in_names.append(partition_name)

    def _body(*args):
        operands = list(args)
        if partition_name is not None:
            operands.append(bass2jax.partition_id_tensor())
        outs = _bass_exec_p.bind(
            *operands,
            out_avals=tuple(out_avals),
            in_names=tuple(all_in_names),
            out_names=tuple(out_names),
            lowering_input_output_aliases=(),
            sim_require_finite=True,
            sim_require_nnan=True,
            nc=nc,
        )
        return tuple(outs)

    devices = jax.devices()[:NCORES]
    mesh = Mesh(np.asarray(devices), ("core",))
    in_specs = (PartitionSpec("core"),) * (n_params + len(out_names))
    out_specs = (PartitionSpec("core"),) * len(out_names)
    fn = jax.jit(shard_map(_body, mesh=mesh, in_specs=in_specs,
                           out_specs=out_specs, check_rep=False),
                 keep_unused=True)

    concat_in = [np.concatenate([in_maps[c][nm] for c in range(NCORES)],
                                axis=0) for nm in in_names]
    concat_zeros = [np.zeros((NCORES * z.shape[0], *z.shape[1:]), z.dtype)
                    for z in zero_outs]
    sh = NamedSharding(mesh, PartitionSpec("core"))
    dev_in = [jax.device_put(a, sh) for a in concat_in + concat_zeros]

    for _ in range(warmup):
        outs = fn(*dev_in)
    jax.block_until_ready(outs)
    # two-point marginal: strips the large fixed per-batch dispatch
    # overhead of the axon proxy from the per-execution estimate
    times = {}
    for it in (iters // 4, iters):
        t0 = time.perf_counter()
        for _ in range(it):
            outs = fn(*dev_in)
        jax.block_until_ready(outs)
        times[it] = time.perf_counter() - t0
    ks = sorted(times)
    return (times[ks[1]] - times[ks[0]]) / (ks[1] - ks[0])


def kernel(**inputs):
    global LAST_RESULTS
    from concourse import bass_utils

    u = np.asarray(inputs["u"], np.float32)
    D_skip = np.asarray(inputs["D_skip"], np.float32)

    nc = _get_nc()
    in_maps = _host_prep(inputs)
    trace = bool(int(os.environ.get("MAMBA_TRACE", "0")))
    res = bass_utils.run_bass_kernel_spmd(
        nc, in_maps, core_ids=list(range(NCORES)), trace=trace)
    LAST_RESULTS = res

    acc = np.zeros((D_MODEL, TOK), np.float32)
    for r in res.results:
        acc += np.asarray(r["y_part"]).astype(np.float32)
    y = acc.T.reshape(BATCH, SEQ, D_MODEL)
    return y + D_skip[None, None, :] * u


# revision 18
# speedup vs baseline: 1.8525x; 1.8525x over previous
"""Mamba mixer (nn_Mixer) Trainium2 Bass kernel, v3.

Sharding: tensor-parallel over d_inner (2048 -> 256 per core, 8 cores).

Structure:
  Phase 1 (per 512-token chunk): in_proj (f32r matmuls, full PE speed),
    causal conv1d as 4 diagonal-matrix PE matmuls, silu (ACT), x_proj
    partial (bf16).  Partials for each batch are DMA'd to DRAM and ONE
    AllReduce per batch (2 total) is fired from the otherwise-empty
    gpsimd queue -- collectives have ~80us fixed latency here, so fewer
    is better, and their latency overlaps phase-1/phase-2 compute.
    silu(z) and conv output xs are spilled to DRAM (SBUF pressure) and
    streamed back in phase 2.
  Phase 2 (per chunk): dt_proj (f32r), softplus, B/C broadcast to 128
    partitions via stride-0 DMA reads of a bf16 DRAM stage (keeps all
    scan elementwise ops in pure-bf16 SBUF for 2x DVE throughput),
    selective scan via hardware tensor_tensor_scan (split across Pool
    and DVE), y accumulation over states in PSUM via identity matmuls,
    gating, out_proj partial -> DRAM (host sums the row-parallel
    partials).

Queues: SP = u loads + y stores + z/xs spill; ACT-HWDGE = phase-2 loads
(xd/bc/z/xs/broadcasts); gpsimd = collectives (plus its share of scans).

Self-contained: hardcodes all shapes; only needs the concourse/bass
runtime that ships in the container.
"""

import os
import numpy as np

# Problem sizes (fixed by the problem statement)
D_MODEL = 1024
D_INNER = 2048
NSTATE = 16
DT_RANK = 64
DCONV = 4
BATCH = 2
SEQ = 4096

NCORES = 8
DS = D_INNER // NCORES          # 256 d_inner rows per core
DT2 = DS // 128                 # 2 partition tiles per core
NXD = DT_RANK + 2 * NSTATE      # 96
LC = 512                        # chunk length (tokens)
CPB = SEQ // LC                 # chunks per batch (8)
NCH = BATCH * CPB               # total chunks (16)
TOK = BATCH * SEQ


def _build_nc(fake_collective=False, phase="all"):
    """Build the Bass program (same SPMD program for all 8 cores).

    fake_collective=True replaces the AllReduce with a local DRAM copy so
    the program is single-core simulable (TimelineSim perf estimates).
    scan_dve_mod: every (col % scan_dve_mod == scan_dve_mod-1) scan runs
    on DVE instead of Pool (engine balance knob).
    w_pool_mod: every (col % w_pool_mod == w_pool_mod-1) w-mul runs on
    Pool instead of DVE.
    """
    import concourse.bass as bass
    import concourse.bacc as bacc
    import concourse.mybir as mybir
    import concourse.tile as tile

    f32 = mybir.dt.float32
    f32r = mybir.dt.float32r
    bf16 = mybir.dt.bfloat16
    AF = mybir.ActivationFunctionType
    OP = mybir.AluOpType

    nc = bacc.Bacc("TRN2", target_bir_lowering=False, debug=False,
                   num_devices=NCORES)

    # ---- kernel I/O (per-core shards prepared on the host) ----
    uT = nc.dram_tensor("uT", [D_MODEL, TOK], f32r, kind="ExternalInput")
    w_in = nc.dram_tensor("w_inT", [D_MODEL, 4 * 128], f32r, kind="ExternalInput")
    conv_diag = nc.dram_tensor("conv_diag", [128, DT2 * DCONV * 128], bf16,
                               kind="ExternalInput")
    conv_b = nc.dram_tensor("conv_b", [128, DT2], f32, kind="ExternalInput")
    w_xp = nc.dram_tensor("w_xpT", [DS, NXD], bf16, kind="ExternalInput")
    w_dt = nc.dram_tensor("w_dtT", [DT_RANK, DS], f32, kind="ExternalInput")
    dt_bias = nc.dram_tensor("dt_bias", [128, DT2], f32, kind="ExternalInput")
    a_neg = nc.dram_tensor("a_neg", [128, DT2 * NSTATE], f32, kind="ExternalInput")
    d_in = nc.dram_tensor("d_in", [128, DT2], f32, kind="ExternalInput")
    w_out = nc.dram_tensor("w_outT", [DS, D_MODEL], bf16, kind="ExternalInput")
    eye_d = nc.dram_tensor("eye128", [128, 128], bf16, kind="ExternalInput")
    y_part = nc.dram_tensor("y_part", [D_MODEL, TOK], bf16, kind="ExternalOutput")

    with tile.TileContext(nc) as tc:
        with (
            tc.tile_pool(name="const", bufs=1) as cpool,
            tc.tile_pool(name="u", bufs=2) as upool,
            tc.tile_pool(name="work", bufs=2) as wpool,
            tc.tile_pool(name="nwork", bufs=2) as npool,
            tc.tile_pool(name="scanio", bufs=6) as siopool,
            tc.tile_pool(name="obuf", bufs=3) as opool,
            tc.tile_pool(name="bcast", bufs=1) as bpool,
            tc.tile_pool(name="mm", bufs=2, space="PSUM") as psmm,
            tc.tile_pool(name="psy", bufs=2, space="PSUM") as psy,
            tc.tile_pool(name="dram", bufs=1, space="DRAM") as dpool,
        ):
            # ---- static weights into SBUF ----
            w_in_sb = cpool.tile([128, 8, 4 * 128], f32r)
            nc.sync.dma_start(w_in_sb[:], w_in.ap().rearrange(
                "(j p) m -> p j m", p=128))
            w_out_sb = cpool.tile([128, DT2, D_MODEL], bf16)
            nc.sync.dma_start(w_out_sb[:], w_out.ap().rearrange(
                "(k p) m -> p k m", p=128))
            w_xp_sb = cpool.tile([128, DT2, NXD], bf16)
            nc.sync.dma_start(w_xp_sb[:], w_xp.ap().rearrange(
                "(k p) m -> p k m", p=128))
            w_dt_sb = cpool.tile([DT_RANK, DS], f32)
            nc.sync.dma_start(w_dt_sb[:], w_dt.ap())
            conv_diag_sb = cpool.tile([128, DT2 * DCONV * 128], bf16)
            nc.sync.dma_start(conv_diag_sb[:], conv_diag.ap())
            conv_b_sb = cpool.tile([128, DT2], f32)
            nc.sync.dma_start(conv_b_sb[:], conv_b.ap())
            dt_bias_sb = cpool.tile([128, DT2], f32)
            nc.sync.dma_start(dt_bias_sb[:], dt_bias.ap())
            a_sb = cpool.tile([128, DT2 * NSTATE], f32)
            nc.sync.dma_start(a_sb[:], a_neg.ap())
            d_in_sb = cpool.tile([128, DT2], f32)
            nc.sync.dma_start(d_in_sb[:], d_in.ap())
            eye_sb = cpool.tile([128, 128], bf16)
            nc.sync.dma_start(eye_sb[:], eye_d.ap())

            # persistent state
            h_all = cpool.tile([128, DT2 * NSTATE, LC], bf16)
            carry = cpool.tile([128, DT2 * NSTATE], f32)

            # DRAM staging
            z_d = dpool.tile([128, DT2 * TOK], bf16, tag="z")
            xs_d = dpool.tile([128, DT2 * TOK], bf16, tag="xs")
            bcd = dpool.tile([2 * NSTATE, TOK], bf16, tag="bcd")
            ar_in = [dpool.tile([NXD, SEQ], f32, tag=f"arin{p}", name=f"arin{p}")
                     for p in range(BATCH)]
            ar_out = [dpool.tile([NXD, SEQ], f32, tag=f"arout{p}", name=f"arout{p}")
                      for p in range(BATCH)]
            z_d3 = z_d[:, :].rearrange("p (k t) -> p k t", k=DT2)
            xs_d3 = xs_d[:, :].rearrange("p (k t) -> p k t", k=DT2)

            uT_ap = uT.ap().rearrange("(j p) t -> p j t", p=128)

            # ================= Phase 1 =================
            p1s = {"x_prev": None}

            def p1_chunk(c):
                x_prev = p1s["x_prev"]
                t0 = c * LC
                u_sb = upool.tile([128, 8, LC], f32r, tag="u")
                nc.scalar.dma_start(u_sb[:], uT_ap[:, :, t0:t0 + LC])

                x_sb = wpool.tile([128, DT2, LC + DCONV - 1], bf16, tag="x")
                if c % CPB == 0:
                    nc.vector.memset(x_sb[:, :, 0:DCONV - 1], 0.0)
                else:
                    nc.vector.tensor_copy(x_sb[:, :, 0:DCONV - 1],
                                          x_prev[:, :, LC:LC + DCONV - 1])
                z_sb = wpool.tile([128, DT2, LC], bf16, tag="z")
                for mt in range(4):
                    ps = psmm.tile([128, LC], f32, tag="mm")
                    for j in range(8):
                        nc.tensor.matmul(
                            ps[:],
                            w_in_sb[:, j, 128 * mt:128 * (mt + 1)],
                            u_sb[:, j, :],
                            start=(j == 0), stop=(j == 7))
                    if mt < DT2:
                        nc.scalar.copy(
                            x_sb[:, mt, DCONV - 1:DCONV - 1 + LC], ps[:])
                    else:
                        nc.scalar.activation(z_sb[:, mt - DT2, :], ps[:],
                                             AF.Silu, bias=0.0)
                nc.sync.dma_start(z_d3[:, :, t0:t0 + LC], z_sb[:])

                # causal conv1d on PE: 4 diagonal matmuls accumulate in PSUM
                xs_sb = wpool.tile([128, DT2, LC], bf16, tag="xs")
                for dt in range(DT2):
                    cps = psy.tile([128, LC], f32, tag=f"y{dt}",
                                   name=f"convps{dt}")
                    for k in range(DCONV):
                        blk = (dt * DCONV + k) * 128
                        nc.tensor.matmul(cps[:],
                                         conv_diag_sb[:, blk:blk + 128],
                                         x_sb[:, dt, k:k + LC],
                                         start=(k == 0), stop=(k == DCONV - 1))
                    nc.scalar.activation(xs_sb[:, dt, :], cps[:],
                                         AF.Silu, bias=conv_b_sb[:, dt:dt + 1])
                nc.sync.dma_start(xs_d3[:, :, t0:t0 + LC], xs_sb[:])

                # x_proj partial
                xp_ps = psmm.tile([NXD, LC], f32, tag="mm", name="xp_ps")
                for dt in range(DT2):
                    nc.tensor.matmul(xp_ps[:], w_xp_sb[:, dt, :],
                                     xs_sb[:, dt, :],
                                     start=(dt == 0), stop=(dt == DT2 - 1))
                xq_sb = wpool.tile([NXD, LC], f32, tag="xq")
                nc.vector.tensor_copy(xq_sb[:], xp_ps[:])
                p = c // CPB
                tp = (c % CPB) * LC
                nc.scalar.dma_start(ar_in[p][:, tp:tp + LC], xq_sb[:])

                if c % CPB == CPB - 1:  # piece (batch) complete -> collective
                    if fake_collective:
                        nc.gpsimd.dma_start(ar_out[p][:], ar_in[p][:])
                    else:
                        nc.gpsimd.collective_compute(
                            "AllReduce", OP.add,
                            replica_groups=[list(range(NCORES))],
                            ins=[ar_in[p].opt()], outs=[ar_out[p].opt()])
                p1s["x_prev"] = x_sb

            # ================= Phase 2 (software-pipelined) ==============
            # stage A(c): loads + dt_proj + softplus + dtx + B/C broadcast
            # stage B(c): a_t exps + dbx muls
            # stage C(c): scans
            # stage D(c): w muls, y matmuls, carry, gate, out_proj, store
            # Emission: A0 B0 | A1 B1 C0 D0 | A2 B2 C1 D1 | ... so each
            # engine queue has chunk c+1 front-stage work before chunk c
            # back-stage work (keeps DVE/Pool from ping-ponging).
            HB = NSTATE // 2
            state = {}

            def stage_A(c):
                p = c // CPB
                t0 = c * LC
                tp = (c % CPB) * LC
                xd_sb = wpool.tile([DT_RANK, LC], f32, tag="xd",
                                   name=f"xd{c}")
                nc.scalar.dma_start(xd_sb[:], ar_out[p][0:DT_RANK, tp:tp + LC])
                bcf = wpool.tile([2 * NSTATE, LC], f32, tag="bcf",
                                 name=f"bcf{c}")
                nc.sync.dma_start(bcf[:], ar_out[p][DT_RANK:NXD, tp:tp + LC])
                bc16 = wpool.tile([2 * NSTATE, LC], bf16, tag="bc16",
                                  name=f"bc16_{c}")
                nc.vector.tensor_copy(bc16[:], bcf[:])
                nc.sync.dma_start(bcd[:, t0:t0 + LC], bc16[:])
                bcB, bcC = [], []
                for hb in range(2):
                    bB = bpool.tile([128, HB, LC], bf16, tag=f"bcB{hb}",
                                    name=f"bcB{hb}_{c}")
                    nc.sync.dma_start(
                        bB[:],
                        bcd[HB * hb:HB * (hb + 1), t0:t0 + LC].rearrange(
                            "(o a) b -> o a b", o=1).broadcast_to(
                                [128, HB, LC]))
                    bcB.append(bB)
                    bC = bpool.tile([128, HB, LC], bf16, tag=f"bcC{hb}",
                                    name=f"bcC{hb}_{c}")
                    nc.sync.dma_start(
                        bC[:],
                        bcd[NSTATE + HB * hb:NSTATE + HB * (hb + 1),
                            t0:t0 + LC].rearrange(
                            "(o a) b -> o a b", o=1).broadcast_to(
                                [128, HB, LC]))
                    bcC.append(bC)
                z_in = wpool.tile([128, DT2, LC], bf16, tag="zin",
                                  name=f"zin{c}")
                nc.scalar.dma_start(z_in[:], z_d3[:, :, t0:t0 + LC])
                xs_in = wpool.tile([128, DT2, LC], bf16, tag="xsin",
                                   name=f"xsin{c}")
                nc.scalar.dma_start(xs_in[:], xs_d3[:, :, t0:t0 + LC])

                dt_sb = wpool.tile([128, DT2, LC], bf16, tag="dt",
                                   name=f"dt{c}")
                dtx = wpool.tile([128, DT2, LC], bf16, tag="dtx",
                                 name=f"dtx{c}")
                dt_ps = psmm.tile([128, DT2, LC], f32, tag="dtmm", bufs=1,
                                  name=f"dtps{c}")
                e_t = npool.tile([128, DT2, LC], f32, tag="esp",
                                 name=f"esp{c}")
                for dt in range(DT2):
                    nc.tensor.matmul(
                        dt_ps[:, dt, :],
                        w_dt_sb[:, 128 * dt:128 * (dt + 1)],
                        xd_sb[:],
                        start=True, stop=True)
                    nc.scalar.activation(e_t[:, dt, :], dt_ps[:, dt, :],
                                         AF.Exp, bias=dt_bias_sb[:, dt:dt + 1])
                nc.scalar.activation(dt_sb[:], e_t[:], AF.Ln, bias=1.0)
                nc.vector.tensor_mul(dtx[:], dt_sb[:], xs_in[:])
                state[c] = dict(dt_sb=dt_sb, dtx=dtx, z_in=z_in,
                                xs_in=xs_in, bcB=bcB, bcC=bcC)

            def stage_B(c):
                st = state[c]
                cols = [(dt, n) for n in range(NSTATE) for dt in range(DT2)]
                a_ts, dbxs = {}, {}
                for dt, n in cols:
                    col = dt * NSTATE + n
                    a_t = siopool.tile([128, LC], bf16, tag="a",
                                       name=f"a{c}_{col}")
                    nc.scalar.activation(a_t[:], st["dt_sb"][:, dt, :],
                                         AF.Exp, bias=0.0,
                                         scale=a_sb[:, col:col + 1])
                    a_ts[col] = a_t
                for dt, n in cols:
                    col = dt * NSTATE + n
                    dbx = siopool.tile([128, LC], bf16, tag="dbx",
                                       name=f"dbx{c}_{col}")
                    dbx_eng = nc.gpsimd if col % 2 == 1 else nc.vector
                    dbx_eng.tensor_mul(dbx[:], st["dtx"][:, dt, :],
                                       st["bcB"][n // HB][:, n % HB, :])
                    dbxs[col] = dbx
                st["a_ts"], st["dbxs"] = a_ts, dbxs

            def stage_C(c):
                st = state[c]
                if c % CPB == 0:
                    nc.vector.memset(carry[:], 0.0)
                cols = [(dt, n) for n in range(NSTATE) for dt in range(DT2)]
                for dt, n in cols:
                    col = dt * NSTATE + n
                    nc.vector.tensor_tensor_scan(
                        h_all[:, col, :], st["a_ts"][col][:],
                        st["dbxs"][col][:],
                        initial=carry[:, col:col + 1],
                        op0=OP.mult, op1=OP.add)

            def stage_D(c):
                st = state.pop(c)
                t0 = c * LC
                cols = [(dt, n) for n in range(NSTATE) for dt in range(DT2)]
                y_ps = [psy.tile([128, LC], f32, tag=f"y{i}",
                                 name=f"y_ps{c}_{i}") for i in range(DT2)]
                w_ts = {}
                for dt, n in cols:
                    col = dt * NSTATE + n
                    w_t = siopool.tile([128, LC], bf16, tag="w",
                                       name=f"w{c}_{col}")
                    w_eng = nc.gpsimd if col % 2 == 0 else nc.vector
                    w_eng.tensor_mul(w_t[:], h_all[:, col, :],
                                     st["bcC"][n // HB][:, n % HB, :])
                    w_ts[col] = w_t
                for n in range(NSTATE):
                    for dt in range(DT2):
                        col = dt * NSTATE + n
                        nc.tensor.matmul(y_ps[dt][:], eye_sb[:],
                                         w_ts[col][:],
                                         start=(n == 0),
                                         stop=(n == NSTATE - 1))
                # carry for next chunk: last column of every scan output
                if c % CPB != CPB - 1:
                    for dt in range(DT2):
                        lo, hi = dt * NSTATE, (dt + 1) * NSTATE
                        nc.scalar.copy(carry[:, lo:hi],
                                       h_all[:, lo:hi, LC - 1])

                # ---- y = y_ssm + D*xs, gate with silu(z), out_proj ----
                yg = wpool.tile([128, DT2, LC], bf16, tag="yg",
                                name=f"yg{c}")
                for dt in range(DT2):
                    ys = npool.tile([128, LC], bf16, tag="ys",
                                    name=f"ys{c}_{dt}")
                    nc.vector.scalar_tensor_tensor(
                        ys[:], st["xs_in"][:, dt, :],
                        d_in_sb[:, dt:dt + 1],
                        y_ps[dt][:], op0=OP.mult, op1=OP.add)
                    nc.vector.tensor_mul(yg[:, dt, :], ys[:],
                                         st["z_in"][:, dt, :])

                for mt in range(8):
                    ps = psmm.tile([128, LC], f32, tag="mm")
                    for kt in range(DT2):
                        nc.tensor.matmul(
                            ps[:],
                            w_out_sb[:, kt, 128 * mt:128 * (mt + 1)],
                            yg[:, kt, :],
                            start=(kt == 0), stop=(kt == DT2 - 1))
                    ob = opool.tile([128, LC], bf16, tag="ob")
                    nc.scalar.copy(ob[:], ps[:])
                    nc.sync.dma_start(
                        y_part[128 * mt:128 * (mt + 1), t0:t0 + LC], ob[:])

            # ---- emission schedule ----
            # batch-0 phase 1 (fires AR0), then a 4-chunk head start on
            # batch-1 phase 1 (covers AR0 latency), then the phase-2
            # pipeline with the remaining phase-1 chunks interleaved
            # (AR1 fires inside iteration 1).
            if phase == "p1":
                for c in range(NCH):
                    p1_chunk(c)
                zb = wpool.tile([128, 8, LC], bf16, tag="zb")
                nc.vector.memset(zb[:], 0.0)
                for c in range(NCH):
                    nc.sync.dma_start(
                        y_part.ap().rearrange("(j p) t -> p j t", p=128)
                        [:, :, c * LC:(c + 1) * LC], zb[:])
            elif phase == "p2":
                stage_A(0)
                stage_B(0)
                for c in range(NCH):
                    if c + 1 < NCH:
                        stage_A(c + 1)
                    stage_C(c)
                    if c + 1 < NCH:
                        stage_B(c + 1)
                    stage_D(c)
            else:
                for c in range(CPB):
                    p1_chunk(c)
                for c in range(CPB, CPB + 4):
                    p1_chunk(c)
                stage_A(0)
                stage_B(0)
                for c in range(NCH):
                    if c < 2:
                        p1_chunk(CPB + 4 + 2 * c)
                        p1_chunk(CPB + 5 + 2 * c)
                    if c + 1 < NCH:
                        stage_A(c + 1)
                    stage_C(c)
                    if c + 1 < NCH:
                        stage_B(c + 1)
                    stage_D(c)

    nc.compile()
    return nc


_CACHED = {}


def _get_nc():
    fake = bool(int(os.environ.get("MAMBA_FAKE_AR", "0")))
    phase = os.environ.get("MAMBA_PHASE", "all")
    key = ("v5", fake, phase)
    if key not in _CACHED:
        _CACHED[key] = _build_nc(fake_collective=fake, phase=phase)
    return _CACHED[key]


def _host_prep(inputs):
    """Slice/transpose the full inputs into per-core in_maps."""
    import ml_dtypes
    _bf = ml_dtypes.bfloat16
    f32 = np.float32
    u = np.asarray(inputs["u"], f32)
    in_proj_w = np.asarray(inputs["in_proj_w"], f32)
    conv_w = np.asarray(inputs["conv_w"], f32)
    conv_b = np.asarray(inputs["conv_b"], f32)
    x_proj_w = np.asarray(inputs["x_proj_w"], f32)
    dt_proj_w = np.asarray(inputs["dt_proj_w"], f32)
    dt_bias = np.asarray(inputs["dt_bias"], f32)
    A_log = np.asarray(inputs["A_log"], f32)
    D_in = np.asarray(inputs["D_in"], f32)
    out_proj_w = np.asarray(inputs["out_proj_w"], f32)

    uT = np.ascontiguousarray(u.reshape(TOK, D_MODEL).T)
    eye = np.eye(128, dtype=f32).astype(_bf)
    A = -np.exp(A_log)

    def fold(v):  # (256, k) -> (128, 2*k) with dtile-major columns
        v = v.reshape(DS, -1)
        return np.ascontiguousarray(
            np.concatenate([v[:128], v[128:]], axis=1))

    in_maps = []
    for k in range(NCORES):
        sl = slice(DS * k, DS * (k + 1))
        w_in_k = np.concatenate(
            [in_proj_w[sl], in_proj_w[D_INNER + DS * k:D_INNER + DS * (k + 1)]])
        cw = fold(conv_w[sl])               # [128, DT2*DCONV]
        cd = np.zeros((128, DT2 * DCONV * 128), f32)
        for dt in range(DT2):
            for kk in range(DCONV):
                blk = (dt * DCONV + kk) * 128
                np.fill_diagonal(cd[:, blk:blk + 128], cw[:, dt * DCONV + kk])
        in_maps.append({
            "uT": uT,
            "w_inT": np.ascontiguousarray(w_in_k.T),
            "conv_diag": cd.astype(_bf),
            "conv_b": fold(conv_b[sl]),
            "w_xpT": np.ascontiguousarray(x_proj_w[:, sl].T).astype(_bf),
            "w_dtT": np.ascontiguousarray(dt_proj_w[sl].T),
            "dt_bias": fold(dt_bias[sl]),
            "a_neg": fold(A[sl]),
            "d_in": fold(D_in[sl]),
            "w_outT": np.ascontiguousarray(out_proj_w[:, sl].T).astype(_bf),
            "eye128": eye,
        })
    return in_maps


LAST_RESULTS = None


def bench(inputs, iters=24, warmup=4):
    """Estimate per-execution device time: device-put the sharded inputs
    once, then dispatch the jitted NEFF repeatedly (async) and time."""
    import time
    import jax
    from jax.sharding import Mesh, PartitionSpec, NamedSharding
    from jax.experimental.shard_map import shard_map
    import concourse.mybir as mybir
    from concourse import bass2jax
    from concourse.bass2jax import _bass_exec_p, install_neuronx_cc_hook

    install_neuronx_cc_hook()
    nc = _get_nc()
    in_maps = _host_prep(inputs)

    partition_name = (nc.partition_id_tensor.name
                      if nc.partition_id_tensor else None)
    in_names, out_names, out_avals, zero_outs = [], [], [], []
    for alloc in nc.m.functions[0].allocations:
        if not isinstance(alloc, mybir.MemoryLocationSet):
            continue
        name = alloc.memorylocations[0].name
        if alloc.kind == "ExternalInput":
            if name != partition_name:
                in_names.append(name)
        elif alloc.kind == "ExternalOutput":
            shape = tuple(alloc.tensor_shape)
            dtype = mybir.dt.np(alloc.dtype)
            out_avals.append(jax.core.ShapedArray(shape, dtype))
            out_names.append(name)
            zero_outs.append(np.zeros(shape, dtype))
    n_params = len(in_names)
    all_in_names = list(in_names) + list(out_names)
    if partition_name is not None:
        all_in_names.append(partition_name)

    def _body(*args):
        operands = list(args)
        if partition_name is not None:
            operands.append(bass2jax.partition_id_tensor())
        outs = _bass_exec_p.bind(
            *operands,
            out_avals=tuple(out_avals),
            in_names=tuple(all_in_names),
            out_names=tuple(out_names),
            lowering_input_output_aliases=(),
            sim_require_finite=True,
            sim_require_nnan=True,
            nc=nc,
        )
        return tuple(outs)

    devices = jax.devices()[:NCORES]
    mesh = Mesh(np.asarray(devices), ("core",))
    in_specs = (PartitionSpec("core"),) * (n_params + len(out_names))
    out_specs = (PartitionSpec("core"),) * len(out_names)
    fn = jax.jit(shard_map(_body, mesh=mesh, in_specs=in_specs,
                           out_specs=out_specs, check_rep=False),
                 keep_unused=True)

    concat_in = [np.concatenate([in_maps[c][nm] for c in range(NCORES)],
                                axis=0) for nm in in_names]
    concat_zeros = [np.zeros((NCORES * z.shape[0], *z.shape[1:]), z.dtype)
                    for z in zero_outs]
    sh = NamedSharding(mesh, PartitionSpec("core"))
    dev_in = [jax.device_put(a, sh) for a in concat_in + concat_zeros]

    for _ in range(warmup):
        outs = fn(*dev_in)
    jax.block_until_ready(outs)
    # two-point marginal: strips the large fixed per-batch dispatch
    # overhead of the axon proxy from the per-execution estimate
    times = {}
    for it in (iters // 4, iters):
        t0 = time.perf_counter()
        for _ in range(it):
            outs = fn(*dev_in)
        jax.block_until_ready(outs)
        times[it] = time.perf_counter() - t0
    ks = sorted(times)
    return (times[ks[1]] - times[ks[0]]) / (ks[1] - ks[0])


def kernel(**inputs):
    global LAST_RESULTS
    from concourse import bass_utils

    u = np.asarray(inputs["u"], np.float32)
    D_skip = np.asarray(inputs["D_skip"], np.float32)

    nc = _get_nc()
    in_maps = _host_prep(inputs)
    trace = bool(int(os.environ.get("MAMBA_TRACE", "0")))
    res = bass_utils.run_bass_kernel_spmd(
        nc, in_maps, core_ids=list(range(NCORES)), trace=trace)
    LAST_RESULTS = res

    acc = np.zeros((D_MODEL, TOK), np.float32)
    for r in res.results:
        acc += np.asarray(r["y_part"]).astype(np.float32)
    y = acc.T.reshape(BATCH, SEQ, D_MODEL)
    return y + D_skip[None, None, :] * u


# revision 19
# speedup vs baseline: 2.0176x; 1.0891x over previous
"""Mamba mixer (nn_Mixer) Trainium2 Bass kernel, v3.

Sharding: tensor-parallel over d_inner (2048 -> 256 per core, 8 cores).

Structure:
  Phase 1 (per 512-token chunk): in_proj (f32r matmuls, full PE speed),
    causal conv1d as 4 diagonal-matrix PE matmuls, silu (ACT), x_proj
    partial (bf16).  Partials for each batch are DMA'd to DRAM and ONE
    AllReduce per batch (2 total) is fired from the otherwise-empty
    gpsimd queue -- collectives have ~80us fixed latency here, so fewer
    is better, and their latency overlaps phase-1/phase-2 compute.
    silu(z) and conv output xs are spilled to DRAM (SBUF pressure) and
    streamed back in phase 2.
  Phase 2 (per chunk): dt_proj (f32r), softplus, B/C broadcast to 128
    partitions via stride-0 DMA reads of a bf16 DRAM stage (keeps all
    scan elementwise ops in pure-bf16 SBUF for 2x DVE throughput),
    selective scan via hardware tensor_tensor_scan (split across Pool
    and DVE), y accumulation over states in PSUM via identity matmuls,
    gating, out_proj partial -> DRAM (host sums the row-parallel
    partials).

Queues: SP = u loads + y stores + z/xs spill; ACT-HWDGE = phase-2 loads
(xd/bc/z/xs/broadcasts); gpsimd = collectives (plus its share of scans).

Self-contained: hardcodes all shapes; only needs the concourse/bass
runtime that ships in the container.
"""

import os
import numpy as np

# Problem sizes (fixed by the problem statement)
D_MODEL = 1024
D_INNER = 2048
NSTATE = 16
DT_RANK = 64
DCONV = 4
BATCH = 2
SEQ = 4096

NCORES = 8
DS = D_INNER // NCORES          # 256 d_inner rows per core
DT2 = DS // 128                 # 2 partition tiles per core
NXD = DT_RANK + 2 * NSTATE      # 96
LC = 512                        # chunk length (tokens)
CPB = SEQ // LC                 # chunks per batch (8)
NCH = BATCH * CPB               # total chunks (16)
TOK = BATCH * SEQ


def _build_nc(fake_collective=False, phase="all", bf16_in=False,
              all_dve=False, no_bcast=False):
    """Build the Bass program (same SPMD program for all 8 cores).

    fake_collective=True replaces the AllReduce with a local DRAM copy so
    the program is single-core simulable (TimelineSim perf estimates).
    scan_dve_mod: every (col % scan_dve_mod == scan_dve_mod-1) scan runs
    on DVE instead of Pool (engine balance knob).
    w_pool_mod: every (col % w_pool_mod == w_pool_mod-1) w-mul runs on
    Pool instead of DVE.
    """
    import concourse.bass as bass
    import concourse.bacc as bacc
    import concourse.mybir as mybir
    import concourse.tile as tile

    f32 = mybir.dt.float32
    f32r = mybir.dt.float32r
    bf16 = mybir.dt.bfloat16
    AF = mybir.ActivationFunctionType
    OP = mybir.AluOpType

    nc = bacc.Bacc("TRN2", target_bir_lowering=False, debug=False,
                   num_devices=NCORES)

    # ---- kernel I/O (per-core shards prepared on the host) ----
    in_dt = bf16 if bf16_in else f32r
    uT = nc.dram_tensor("uT", [D_MODEL, TOK], in_dt, kind="ExternalInput")
    w_in = nc.dram_tensor("w_inT", [D_MODEL, 4 * 128], in_dt, kind="ExternalInput")
    conv_diag = nc.dram_tensor("conv_diag", [128, DT2 * DCONV * 128], bf16,
                               kind="ExternalInput")
    conv_b = nc.dram_tensor("conv_b", [128, DT2], f32, kind="ExternalInput")
    w_xp = nc.dram_tensor("w_xpT", [DS, NXD], bf16, kind="ExternalInput")
    w_dt = nc.dram_tensor("w_dtT", [DT_RANK, DS], f32, kind="ExternalInput")
    dt_bias = nc.dram_tensor("dt_bias", [128, DT2], f32, kind="ExternalInput")
    a_neg = nc.dram_tensor("a_neg", [128, DT2 * NSTATE], f32, kind="ExternalInput")
    d_in = nc.dram_tensor("d_in", [128, DT2], f32, kind="ExternalInput")
    w_out = nc.dram_tensor("w_outT", [DS, D_MODEL], bf16, kind="ExternalInput")
    eye_d = nc.dram_tensor("eye128", [128, 128], bf16, kind="ExternalInput")
    y_part = nc.dram_tensor("y_part", [D_MODEL, TOK], bf16, kind="ExternalOutput")

    with tile.TileContext(nc) as tc:
        with (
            tc.tile_pool(name="const", bufs=1) as cpool,
            tc.tile_pool(name="u", bufs=2) as upool,
            tc.tile_pool(name="work", bufs=2) as wpool,
            tc.tile_pool(name="nwork", bufs=2) as npool,
            tc.tile_pool(name="scanio", bufs=6) as siopool,
            tc.tile_pool(name="obuf", bufs=3) as opool,
            tc.tile_pool(name="bcast", bufs=1) as bpool,
            tc.tile_pool(name="mm", bufs=2, space="PSUM") as psmm,
            tc.tile_pool(name="psy", bufs=2, space="PSUM") as psy,
            tc.tile_pool(name="dram", bufs=1, space="DRAM") as dpool,
        ):
            # ---- static weights into SBUF ----
            w_in_sb = cpool.tile([128, 8, 4 * 128], in_dt)
            nc.sync.dma_start(w_in_sb[:], w_in.ap().rearrange(
                "(j p) m -> p j m", p=128))
            w_out_sb = cpool.tile([128, DT2, D_MODEL], bf16)
            nc.sync.dma_start(w_out_sb[:], w_out.ap().rearrange(
                "(k p) m -> p k m", p=128))
            w_xp_sb = cpool.tile([128, DT2, NXD], bf16)
            nc.sync.dma_start(w_xp_sb[:], w_xp.ap().rearrange(
                "(k p) m -> p k m", p=128))
            w_dt_sb = cpool.tile([DT_RANK, DS], f32)
            nc.sync.dma_start(w_dt_sb[:], w_dt.ap())
            conv_diag_sb = cpool.tile([128, DT2 * DCONV * 128], bf16)
            nc.sync.dma_start(conv_diag_sb[:], conv_diag.ap())
            conv_b_sb = cpool.tile([128, DT2], f32)
            nc.sync.dma_start(conv_b_sb[:], conv_b.ap())
            dt_bias_sb = cpool.tile([128, DT2], f32)
            nc.sync.dma_start(dt_bias_sb[:], dt_bias.ap())
            a_sb = cpool.tile([128, DT2 * NSTATE], f32)
            nc.sync.dma_start(a_sb[:], a_neg.ap())
            d_in_sb = cpool.tile([128, DT2], f32)
            nc.sync.dma_start(d_in_sb[:], d_in.ap())
            eye_sb = cpool.tile([128, 128], bf16)
            nc.sync.dma_start(eye_sb[:], eye_d.ap())

            # persistent state
            h_all = cpool.tile([128, DT2 * NSTATE, LC], bf16)
            carry = cpool.tile([128, DT2 * NSTATE], f32)

            # DRAM staging
            z_d = dpool.tile([128, DT2 * TOK], bf16, tag="z")
            xs_d = dpool.tile([128, DT2 * TOK], bf16, tag="xs")
            bcd = dpool.tile([2 * NSTATE, TOK], bf16, tag="bcd")
            ar_in = [dpool.tile([NXD, SEQ], f32, tag=f"arin{p}", name=f"arin{p}")
                     for p in range(BATCH)]
            ar_out = [dpool.tile([NXD, SEQ], f32, tag=f"arout{p}", name=f"arout{p}")
                      for p in range(BATCH)]
            z_d3 = z_d[:, :].rearrange("p (k t) -> p k t", k=DT2)
            xs_d3 = xs_d[:, :].rearrange("p (k t) -> p k t", k=DT2)

            uT_ap = uT.ap().rearrange("(j p) t -> p j t", p=128)

            # ================= Phase 1 =================
            p1s = {"x_prev": None}

            def p1_chunk(c):
                x_prev = p1s["x_prev"]
                t0 = c * LC
                u_sb = upool.tile([128, 8, LC], in_dt, tag="u")
                nc.scalar.dma_start(u_sb[:], uT_ap[:, :, t0:t0 + LC])

                x_sb = wpool.tile([128, DT2, LC + DCONV - 1], bf16, tag="x")
                if c % CPB == 0:
                    nc.vector.memset(x_sb[:, :, 0:DCONV - 1], 0.0)
                else:
                    nc.vector.tensor_copy(x_sb[:, :, 0:DCONV - 1],
                                          x_prev[:, :, LC:LC + DCONV - 1])
                z_sb = wpool.tile([128, DT2, LC], bf16, tag="z")
                for mt in range(4):
                    ps = psmm.tile([128, LC], f32, tag="mm")
                    for j in range(8):
                        nc.tensor.matmul(
                            ps[:],
                            w_in_sb[:, j, 128 * mt:128 * (mt + 1)],
                            u_sb[:, j, :],
                            start=(j == 0), stop=(j == 7))
                    if mt < DT2:
                        nc.scalar.copy(
                            x_sb[:, mt, DCONV - 1:DCONV - 1 + LC], ps[:])
                    else:
                        nc.scalar.activation(z_sb[:, mt - DT2, :], ps[:],
                                             AF.Silu, bias=0.0)
                nc.sync.dma_start(z_d3[:, :, t0:t0 + LC], z_sb[:])

                # causal conv1d on PE: 4 diagonal matmuls accumulate in PSUM
                xs_sb = wpool.tile([128, DT2, LC], bf16, tag="xs")
                for dt in range(DT2):
                    cps = psy.tile([128, LC], f32, tag=f"y{dt}",
                                   name=f"convps{dt}")
                    for k in range(DCONV):
                        blk = (dt * DCONV + k) * 128
                        nc.tensor.matmul(cps[:],
                                         conv_diag_sb[:, blk:blk + 128],
                                         x_sb[:, dt, k:k + LC],
                                         start=(k == 0), stop=(k == DCONV - 1))
                    nc.scalar.activation(xs_sb[:, dt, :], cps[:],
                                         AF.Silu, bias=conv_b_sb[:, dt:dt + 1])
                nc.sync.dma_start(xs_d3[:, :, t0:t0 + LC], xs_sb[:])

                # x_proj partial
                xp_ps = psmm.tile([NXD, LC], f32, tag="mm", name="xp_ps")
                for dt in range(DT2):
                    nc.tensor.matmul(xp_ps[:], w_xp_sb[:, dt, :],
                                     xs_sb[:, dt, :],
                                     start=(dt == 0), stop=(dt == DT2 - 1))
                xq_sb = wpool.tile([NXD, LC], f32, tag="xq")
                nc.vector.tensor_copy(xq_sb[:], xp_ps[:])
                p = c // CPB
                tp = (c % CPB) * LC
                nc.scalar.dma_start(ar_in[p][:, tp:tp + LC], xq_sb[:])

                if c % CPB == CPB - 1:  # piece (batch) complete -> collective
                    if fake_collective:
                        nc.gpsimd.dma_start(ar_out[p][:], ar_in[p][:])
                    else:
                        nc.gpsimd.collective_compute(
                            "AllReduce", OP.add,
                            replica_groups=[list(range(NCORES))],
                            ins=[ar_in[p].opt()], outs=[ar_out[p].opt()])
                p1s["x_prev"] = x_sb

            # ================= Phase 2 (software-pipelined) ==============
            # stage A(c): loads + dt_proj + softplus + dtx + B/C broadcast
            # stage B(c): a_t exps + dbx muls
            # stage C(c): scans
            # stage D(c): w muls, y matmuls, carry, gate, out_proj, store
            # Emission: A0 B0 | A1 B1 C0 D0 | A2 B2 C1 D1 | ... so each
            # engine queue has chunk c+1 front-stage work before chunk c
            # back-stage work (keeps DVE/Pool from ping-ponging).
            HB = NSTATE // 2
            state = {}

            def stage_A(c):
                p = c // CPB
                t0 = c * LC
                tp = (c % CPB) * LC
                xd_sb = wpool.tile([DT_RANK, LC], f32, tag="xd",
                                   name=f"xd{c}")
                nc.scalar.dma_start(xd_sb[:], ar_out[p][0:DT_RANK, tp:tp + LC])
                bcf = wpool.tile([2 * NSTATE, LC], f32, tag="bcf",
                                 name=f"bcf{c}")
                nc.sync.dma_start(bcf[:], ar_out[p][DT_RANK:NXD, tp:tp + LC])
                bc16 = wpool.tile([2 * NSTATE, LC], bf16, tag="bc16",
                                  name=f"bc16_{c}")
                nc.vector.tensor_copy(bc16[:], bcf[:])
                nc.sync.dma_start(bcd[:, t0:t0 + LC], bc16[:])
                bcB, bcC = [], []
                if no_bcast:
                    if "const_bc" not in p1s:
                        cb = cpool.tile([128, 2 * NSTATE, LC], bf16,
                                        name="constbc")
                        nc.vector.memset(cb[:], 0.25)
                        p1s["const_bc"] = cb
                    cb = p1s["const_bc"]
                    bcB = [cb[:, 0:HB, :], cb[:, HB:NSTATE, :]]
                    bcC = [cb[:, NSTATE:NSTATE + HB, :],
                           cb[:, NSTATE + HB:2 * NSTATE, :]]
                for hb in range(2 if not no_bcast else 0):
                    bB = bpool.tile([128, HB, LC], bf16, tag=f"bcB{hb}",
                                    name=f"bcB{hb}_{c}")
                    nc.sync.dma_start(
                        bB[:],
                        bcd[HB * hb:HB * (hb + 1), t0:t0 + LC].rearrange(
                            "(o a) b -> o a b", o=1).broadcast_to(
                                [128, HB, LC]))
                    bcB.append(bB)
                    bC = bpool.tile([128, HB, LC], bf16, tag=f"bcC{hb}",
                                    name=f"bcC{hb}_{c}")
                    nc.sync.dma_start(
                        bC[:],
                        bcd[NSTATE + HB * hb:NSTATE + HB * (hb + 1),
                            t0:t0 + LC].rearrange(
                            "(o a) b -> o a b", o=1).broadcast_to(
                                [128, HB, LC]))
                    bcC.append(bC)
                z_in = wpool.tile([128, DT2, LC], bf16, tag="zin",
                                  name=f"zin{c}")
                nc.scalar.dma_start(z_in[:], z_d3[:, :, t0:t0 + LC])
                xs_in = wpool.tile([128, DT2, LC], bf16, tag="xsin",
                                   name=f"xsin{c}")
                nc.scalar.dma_start(xs_in[:], xs_d3[:, :, t0:t0 + LC])

                dt_sb = wpool.tile([128, DT2, LC], bf16, tag="dt",
                                   name=f"dt{c}")
                dtx = wpool.tile([128, DT2, LC], bf16, tag="dtx",
                                 name=f"dtx{c}")
                dt_ps = psmm.tile([128, DT2, LC], f32, tag="dtmm", bufs=1,
                                  name=f"dtps{c}")
                e_t = npool.tile([128, DT2, LC], f32, tag="esp",
                                 name=f"esp{c}")
                for dt in range(DT2):
                    nc.tensor.matmul(
                        dt_ps[:, dt, :],
                        w_dt_sb[:, 128 * dt:128 * (dt + 1)],
                        xd_sb[:],
                        start=True, stop=True)
                    nc.scalar.activation(e_t[:, dt, :], dt_ps[:, dt, :],
                                         AF.Exp, bias=dt_bias_sb[:, dt:dt + 1])
                nc.scalar.activation(dt_sb[:], e_t[:], AF.Ln, bias=1.0)
                nc.vector.tensor_mul(dtx[:], dt_sb[:], xs_in[:])
                state[c] = dict(dt_sb=dt_sb, dtx=dtx, z_in=z_in,
                                xs_in=xs_in, bcB=bcB, bcC=bcC)

            def stage_B(c):
                st = state[c]
                cols = [(dt, n) for n in range(NSTATE) for dt in range(DT2)]
                a_ts, dbxs = {}, {}
                for dt, n in cols:
                    col = dt * NSTATE + n
                    a_t = siopool.tile([128, LC], bf16, tag="a",
                                       name=f"a{c}_{col}")
                    nc.scalar.activation(a_t[:], st["dt_sb"][:, dt, :],
                                         AF.Exp, bias=0.0,
                                         scale=a_sb[:, col:col + 1])
                    a_ts[col] = a_t
                for dt, n in cols:
                    col = dt * NSTATE + n
                    dbx = siopool.tile([128, LC], bf16, tag="dbx",
                                       name=f"dbx{c}_{col}")
                    dbx_eng = (nc.vector if all_dve else
                               (nc.gpsimd if col % 2 == 1 else nc.vector))
                    dbx_eng.tensor_mul(dbx[:], st["dtx"][:, dt, :],
                                       st["bcB"][n // HB][:, n % HB, :])
                    dbxs[col] = dbx
                st["a_ts"], st["dbxs"] = a_ts, dbxs

            def stage_C(c):
                st = state[c]
                if c % CPB == 0:
                    nc.vector.memset(carry[:], 0.0)
                cols = [(dt, n) for n in range(NSTATE) for dt in range(DT2)]
                for dt, n in cols:
                    col = dt * NSTATE + n
                    nc.vector.tensor_tensor_scan(
                        h_all[:, col, :], st["a_ts"][col][:],
                        st["dbxs"][col][:],
                        initial=carry[:, col:col + 1],
                        op0=OP.mult, op1=OP.add)

            def stage_D(c):
                st = state.pop(c)
                t0 = c * LC
                cols = [(dt, n) for n in range(NSTATE) for dt in range(DT2)]
                y_ps = [psy.tile([128, LC], f32, tag=f"y{i}",
                                 name=f"y_ps{c}_{i}") for i in range(DT2)]
                w_ts = {}
                for dt, n in cols:
                    col = dt * NSTATE + n
                    w_t = siopool.tile([128, LC], bf16, tag="w",
                                       name=f"w{c}_{col}")
                    w_eng = (nc.vector if all_dve else
                             (nc.gpsimd if col % 2 == 0 else nc.vector))
                    w_eng.tensor_mul(w_t[:], h_all[:, col, :],
                                     st["bcC"][n // HB][:, n % HB, :])
                    w_ts[col] = w_t
                for n in range(NSTATE):
                    for dt in range(DT2):
                        col = dt * NSTATE + n
                        nc.tensor.matmul(y_ps[dt][:], eye_sb[:],
                                         w_ts[col][:],
                                         start=(n == 0),
                                         stop=(n == NSTATE - 1))
                # carry for next chunk: last column of every scan output
                if c % CPB != CPB - 1:
                    for dt in range(DT2):
                        lo, hi = dt * NSTATE, (dt + 1) * NSTATE
                        nc.scalar.copy(carry[:, lo:hi],
                                       h_all[:, lo:hi, LC - 1])

                # ---- y = y_ssm + D*xs, gate with silu(z), out_proj ----
                yg = wpool.tile([128, DT2, LC], bf16, tag="yg",
                                name=f"yg{c}")
                for dt in range(DT2):
                    ys = npool.tile([128, LC], bf16, tag="ys",
                                    name=f"ys{c}_{dt}")
                    nc.vector.scalar_tensor_tensor(
                        ys[:], st["xs_in"][:, dt, :],
                        d_in_sb[:, dt:dt + 1],
                        y_ps[dt][:], op0=OP.mult, op1=OP.add)
                    nc.vector.tensor_mul(yg[:, dt, :], ys[:],
                                         st["z_in"][:, dt, :])

                for mt in range(8):
                    ps = psmm.tile([128, LC], f32, tag="mm")
                    for kt in range(DT2):
                        nc.tensor.matmul(
                            ps[:],
                            w_out_sb[:, kt, 128 * mt:128 * (mt + 1)],
                            yg[:, kt, :],
                            start=(kt == 0), stop=(kt == DT2 - 1))
                    ob = opool.tile([128, LC], bf16, tag="ob")
                    nc.scalar.copy(ob[:], ps[:])
                    nc.sync.dma_start(
                        y_part[128 * mt:128 * (mt + 1), t0:t0 + LC], ob[:])

            # ---- emission schedule ----
            # batch-0 phase 1 (fires AR0), then a 4-chunk head start on
            # batch-1 phase 1 (covers AR0 latency), then the phase-2
            # pipeline with the remaining phase-1 chunks interleaved
            # (AR1 fires inside iteration 1).
            if phase == "p1":
                for c in range(NCH):
                    p1_chunk(c)
                zb = wpool.tile([128, 8, LC], bf16, tag="zb")
                nc.vector.memset(zb[:], 0.0)
                for c in range(NCH):
                    nc.sync.dma_start(
                        y_part.ap().rearrange("(j p) t -> p j t", p=128)
                        [:, :, c * LC:(c + 1) * LC], zb[:])
            elif phase == "p2":
                stage_A(0)
                stage_B(0)
                for c in range(NCH):
                    if c + 1 < NCH:
                        stage_A(c + 1)
                    stage_C(c)
                    if c + 1 < NCH:
                        stage_B(c + 1)
                    stage_D(c)
            else:
                for c in range(CPB):
                    p1_chunk(c)
                for c in range(CPB, CPB + 4):
                    p1_chunk(c)
                stage_A(0)
                stage_B(0)
                for c in range(NCH):
                    if c < 2:
                        p1_chunk(CPB + 4 + 2 * c)
                        p1_chunk(CPB + 5 + 2 * c)
                    if c + 1 < NCH:
                        stage_A(c + 1)
                    stage_C(c)
                    if c + 1 < NCH:
                        stage_B(c + 1)
                    stage_D(c)

    nc.compile()
    return nc


_CACHED = {}


def _env_knobs():
    return dict(
        fake_collective=bool(int(os.environ.get("MAMBA_FAKE_AR", "0"))),
        phase=os.environ.get("MAMBA_PHASE", "all"),
        bf16_in=bool(int(os.environ.get("MAMBA_BF16_IN", "0"))),
        all_dve=bool(int(os.environ.get("MAMBA_ALL_DVE", "0"))),
        no_bcast=bool(int(os.environ.get("MAMBA_NO_BCAST", "0"))),
    )


def _get_nc():
    kw = _env_knobs()
    key = ("v5",) + tuple(sorted(kw.items()))
    if key not in _CACHED:
        _CACHED[key] = _build_nc(**kw)
    return _CACHED[key]


def _host_prep(inputs):
    """Slice/transpose the full inputs into per-core in_maps."""
    import ml_dtypes
    _bf = ml_dtypes.bfloat16
    f32 = np.float32
    u = np.asarray(inputs["u"], f32)
    in_proj_w = np.asarray(inputs["in_proj_w"], f32)
    conv_w = np.asarray(inputs["conv_w"], f32)
    conv_b = np.asarray(inputs["conv_b"], f32)
    x_proj_w = np.asarray(inputs["x_proj_w"], f32)
    dt_proj_w = np.asarray(inputs["dt_proj_w"], f32)
    dt_bias = np.asarray(inputs["dt_bias"], f32)
    A_log = np.asarray(inputs["A_log"], f32)
    D_in = np.asarray(inputs["D_in"], f32)
    out_proj_w = np.asarray(inputs["out_proj_w"], f32)

    uT = np.ascontiguousarray(u.reshape(TOK, D_MODEL).T)
    eye = np.eye(128, dtype=f32).astype(_bf)
    A = -np.exp(A_log)

    def fold(v):  # (256, k) -> (128, 2*k) with dtile-major columns
        v = v.reshape(DS, -1)
        return np.ascontiguousarray(
            np.concatenate([v[:128], v[128:]], axis=1))

    in_maps = []
    for k in range(NCORES):
        sl = slice(DS * k, DS * (k + 1))
        w_in_k = np.concatenate(
            [in_proj_w[sl], in_proj_w[D_INNER + DS * k:D_INNER + DS * (k + 1)]])
        cw = fold(conv_w[sl])               # [128, DT2*DCONV]
        cd = np.zeros((128, DT2 * DCONV * 128), f32)
        for dt in range(DT2):
            for kk in range(DCONV):
                blk = (dt * DCONV + kk) * 128
                np.fill_diagonal(cd[:, blk:blk + 128], cw[:, dt * DCONV + kk])
        bf16_in = bool(int(os.environ.get("MAMBA_BF16_IN", "0")))
        in_maps.append({
            "uT": uT.astype(_bf) if bf16_in else uT,
            "w_inT": (np.ascontiguousarray(w_in_k.T).astype(_bf)
                      if bf16_in else np.ascontiguousarray(w_in_k.T)),
            "conv_diag": cd.astype(_bf),
            "conv_b": fold(conv_b[sl]),
            "w_xpT": np.ascontiguousarray(x_proj_w[:, sl].T).astype(_bf),
            "w_dtT": np.ascontiguousarray(dt_proj_w[sl].T),
            "dt_bias": fold(dt_bias[sl]),
            "a_neg": fold(A[sl]),
            "d_in": fold(D_in[sl]),
            "w_outT": np.ascontiguousarray(out_proj_w[:, sl].T).astype(_bf),
            "eye128": eye,
        })
    return in_maps


LAST_RESULTS = None


def bench(inputs, iters=24, warmup=4):
    """Estimate per-execution device time: device-put the sharded inputs
    once, then dispatch the jitted NEFF repeatedly (async) and time."""
    import time
    import jax
    from jax.sharding import Mesh, PartitionSpec, NamedSharding
    from jax.experimental.shard_map import shard_map
    import concourse.mybir as mybir
    from concourse import bass2jax
    from concourse.bass2jax import _bass_exec_p, install_neuronx_cc_hook

    install_neuronx_cc_hook()
    nc = _get_nc()
    in_maps = _host_prep(inputs)

    partition_name = (nc.partition_id_tensor.name
                      if nc.partition_id_tensor else None)
    in_names, out_names, out_avals, zero_outs = [], [], [], []
    for alloc in nc.m.functions[0].allocations:
        if not isinstance(alloc, mybir.MemoryLocationSet):
            continue
        name = alloc.memorylocations[0].name
        if alloc.kind == "ExternalInput":
            if name != partition_name:
                in_names.append(name)
        elif alloc.kind == "ExternalOutput":
            shape = tuple(alloc.tensor_shape)
            dtype = mybir.dt.np(alloc.dtype)
            out_avals.append(jax.core.ShapedArray(shape, dtype))
            out_names.append(name)
            zero_outs.append(np.zeros(shape, dtype))
    n_params = len(in_names)
    all_in_names = list(in_names) + list(out_names)
    if partition_name is not None:
        all_in_names.append(partition_name)

    def _body(*args):
        operands = list(args)
        if partition_name is not None:
            operands.append(bass2jax.partition_id_tensor())
        outs = _bass_exec_p.bind(
            *operands,
            out_avals=tuple(out_avals),
            in_names=tuple(all_in_names),
            out_names=tuple(out_names),
            lowering_input_output_aliases=(),
            sim_require_finite=True,
            sim_require_nnan=True,
            nc=nc,
        )
        return tuple(outs)

    devices = jax.devices()[:NCORES]
    mesh = Mesh(np.asarray(devices), ("core",))
    in_specs = (PartitionSpec("core"),) * (n_params + len(out_names))
    out_specs = (PartitionSpec("core"),) * len(out_names)
    fn = jax.jit(shard_map(_body, mesh=mesh, in_specs=in_specs,
                           out_specs=out_specs, check_rep=False),
                 keep_unused=True)

    concat_in = [np.concatenate([in_maps[c][nm] for c in range(NCORES)],
                                axis=0) for nm in in_names]
    concat_zeros = [np.zeros((NCORES * z.shape[0], *z.shape[1:]), z.dtype)
                    for z in zero_outs]
    sh = NamedSharding(mesh, PartitionSpec("core"))
    dev_in = [jax.device_put(a, sh) for a in concat_in + concat_zeros]

    for _ in range(warmup):
        outs = fn(*dev_in)
    jax.block_until_ready(outs)
    # two-point marginal: strips the large fixed per-batch dispatch
    # overhead of the axon proxy from the per-execution estimate
    times = {}
    for it in (iters // 4, iters):
        t0 = time.perf_counter()
        for _ in range(it):
            outs = fn(*dev_in)
        jax.block_until_ready(outs)
        times[it] = time.perf_counter() - t0
    ks = sorted(times)
    return (times[ks[1]] - times[ks[0]]) / (ks[1] - ks[0])


def kernel(**inputs):
    global LAST_RESULTS
    from concourse import bass_utils

    u = np.asarray(inputs["u"], np.float32)
    D_skip = np.asarray(inputs["D_skip"], np.float32)

    nc = _get_nc()
    in_maps = _host_prep(inputs)
    trace = bool(int(os.environ.get("MAMBA_TRACE", "0")))
    res = bass_utils.run_bass_kernel_spmd(
        nc, in_maps, core_ids=list(range(NCORES)), trace=trace)
    LAST_RESULTS = res

    acc = np.zeros((D_MODEL, TOK), np.float32)
    for r in res.results:
        acc += np.asarray(r["y_part"]).astype(np.float32)
    y = acc.T.reshape(BATCH, SEQ, D_MODEL)
    return y + D_skip[None, None, :] * u
